# revision 34
# baseline (speedup 1.0000x reference)
"""GATv2-style masked attention kernel for Trainium2, 8-core data-parallel over batch.

Per core (one batch element, N=2048 nodes, F=256 features):
  h = x @ W                              (PE, fp16)
  s_src = h @ a[:F], s_dst = h @ a[F:]   (PE, fused into the same matmuls)
  e[i,j] = leaky_relu(s_src[i] + s_dst[j], 0.2), masked by A
  alpha = softmax_j(e); y = alpha @ h

Softmax without row maxima: any per-i factor cancels in the normalization
y = (P @ [h|1]) -> y[:, :F] / y[:, F].  Since exp is monotone,
  P[j,i] = exp(max(-0.8*s_src_i, 0.8*s_dst_j) + 0.2*s_dst_j - 54)
         = max(E_i, F_j) * G_j
with E_i = exp(-0.8*s_src_i - 27), F_j = exp(0.8*s_dst_j - 27),
G_j = exp(0.2*s_dst_j - 27): the N x N exp stream disappears entirely --
only per-node exp vectors are needed.  Per score chunk the DVE computes
(E max F_j) * G_j as one 4x-mode tensor_scalar, then the {0,1} mask is
applied with one 2x tensor_tensor; some mask multiplies run on the
otherwise-idle GPSIMD engine.  The -54 recentering (3.4*sigma with
sigma = ||W @ a_dst|| ~= 16 for this randn input spec) keeps the bf16
score tiles centered where the big softmax weights live.

Scores are built transposed ([j, i]) so the P @ h contraction has j on
partitions.  The i range is processed in two waves of 8 PSUM banks each.
h is produced two i-chunks per PSUM bank ([W] only); s_dst accumulates in a
16-column bank via rank-1 matmuls against the w_dst column so the F/G exps
batch into four activations (the F/G exps are emitted before the E2 seg
drains so the DVE strip stream starts as early as possible); h->SBUF drains
are spread over ACT and DVE (GPSIMD cannot touch PSUM) so the banks recycle
quickly into the wave accumulators.  The last STAG j-strips of each wave
run bank-major so normalization and the bf16 output stores pipeline behind
the matmul tail instead of serializing after it.  A stream of tiny junk
matmuls at t=0 rides out the PE p-state ramp so the real matmul stream
runs at full clock.

The E_i replication across partitions comes from a rank-1 matmul whose
stationary operand is the w_src column read through a stride-0
broadcast_to AP, so no replicated weight tensor is ever transferred.  The
host packs [W | W@a_dst | W@a_src | x^T] into one fp16 tensor so the first
DMA (whose completion semaphore alone gates the first real matmul) carries
the weights plus the first x columns; the mask is transposed bf16 {0,1}
(pure layout/precision transforms); y is stored bf16 and upcast on the
host.
"""

import os
import numpy as np

B, N, F = 8, 2048, 256
PC = N // 128        # 16 j-chunks
CC = 2               # contraction chunks (f in two halves of 128)
HALF = N // 2
NWARM = int(os.environ.get("K_NWARM", "48"))
STAG = int(os.environ.get("K_STAG", "6"))  # last STAG j-strips run bank-major
_CACHE = {}


def _build():
    if "nc" in _CACHE:
        return _CACHE["nc"]

    from contextlib import ExitStack
    import concourse.bacc as bacc
    import concourse.tile as tile
    import concourse.mybir as mybir

    dt = mybir.dt
    AF = mybir.ActivationFunctionType
    ALU = mybir.AluOpType

    nc = bacc.Bacc("TRN2", target_bir_lowering=False, debug=False, num_devices=B)

    # xW packs [Wsd | xT]: host-side concat so the first DMA carries both the
    # weights and x quarter 0 (one completion sem gates the first matmul).
    xW = nc.dram_tensor("xW", [F, F + 2 + N], dt.float16, kind="ExternalInput").ap()
    maskT = nc.dram_tensor("maskT", [N, N], dt.bfloat16, kind="ExternalInput").ap()
    y = nc.dram_tensor("y", [N, F], dt.bfloat16, kind="ExternalOutput").ap()

    with tile.TileContext(nc) as tc, ExitStack() as ctx:
        sb = ctx.enter_context(tc.tile_pool(name="sb", bufs=1))
        tpool = ctx.enter_context(tc.tile_pool(name="tp", bufs=int(os.environ.get("K_TP", "6"))))
        phpool = ctx.enter_context(tc.tile_pool(name="ph", bufs=int(os.environ.get("K_PH", "8"))))
        ypool = ctx.enter_context(tc.tile_pool(name="ysb", bufs=8))
        spool = ctx.enter_context(tc.tile_pool(name="small", bufs=4))
        ps = ctx.enter_context(tc.tile_pool(name="ps", bufs=8, space="PSUM"))

        # ---- persistent SBUF tensors ----
        WX = F + 2 + 1024  # [Wsd | x half 0] per contraction chunk
        xw0 = sb.tile([128, CC, WX], dt.float16, tag="xw0")
        xh1 = sb.tile([128, CC, 1024], dt.float16, tag="xh1")
        Wsd_t = xw0[:, :, 0 : F + 2]
        xh = [xw0[:, :, F + 2 : WX], xh1]
        maskS = [
            sb.tile([128, HALF], dt.bfloat16, tag=f"maskS{j}", name=f"maskS{j}")
            for j in range(2 * PC)
        ]
        hh = sb.tile([128, PC, F + 2], dt.float16, tag="hh")    # [h | 1] per chunk
        E2 = [
            sb.tile([128, HALF], dt.bfloat16, tag=f"E2_{i}", name=f"E2_{i}")
            for i in range(2)
        ]  # exp(-0.8*s_src - 27) replicated, per i-half
        Fv = sb.tile([128, PC], dt.float32, tag="Fv")  # exp(0.8*s_dst - 27)
        Gv = sb.tile([128, PC], dt.float32, tag="Gv")  # exp(0.2*s_dst - 27)
        cst = sb.tile([128, 4], dt.float32, tag="cst")  # -27, -0.8, 0.8, 0.2
        junk = sb.tile([128, 64], dt.float16, tag="junk")

        nc.gpsimd.memset(junk[:], 0.0)
        nc.gpsimd.memset(cst[:, 0:1], -27.0)
        nc.gpsimd.memset(cst[:, 1:2], -0.8)
        nc.gpsimd.memset(cst[:, 2:3], 0.8)
        nc.gpsimd.memset(cst[:, 3:4], 0.2)
        nc.vector.memset(hh[:, :, F : F + 1], 1.0)
        # Pull the Exp activation table in before the DMA window closes.
        scr = spool.tile([128, 1], dt.float32, tag="rec", name="scr")
        nc.scalar.activation(scr[:], cst[:, 0:1], AF.Exp, bias=0.0, scale=1.0)

        # ---- PE p-state warm-up ----
        warm = ps.tile([64, 64], dt.float32, tag="bank", name="warm")
        for _ in range(NWARM):
            nc.tensor.matmul(warm[:], junk[:, 0:64], junk[:, 0:64], start=True, stop=True)

        # ---- DMAs ----
        xWr = xW.rearrange("(c p) n -> p c n", p=128)

        def load_mask(w, j):
            nc.sync.dma_start(
                maskS[w * PC + j][:],
                maskT[j * 128 : (j + 1) * 128, w * HALF : (w + 1) * HALF],
            )

        def load_xcols(lo, hi):
            # lo/hi in xW column space; lo==0 also carries Wsd
            h = 0 if hi <= F + 2 + 1024 else 1
            dst = xw0 if h == 0 else xh1
            off = lo if h == 0 else lo - (F + 2 + 1024)
            nc.sync.dma_start(
                dst[:, :, off : off + (hi - lo)], xWr[:, :, lo:hi]
            )

        W2 = F + 2
        load_xcols(0, W2 + 256)
        load_xcols(W2 + 256, W2 + 512)

        def load_rest():
            load_xcols(W2 + 512, W2 + 1024)
            load_xcols(W2 + 1024, W2 + 1536)
            load_mask(0, 0)
            load_xcols(W2 + 1536, W2 + 2048)
            load_mask(0, 1)
            for j in range(2, PC):
                load_mask(0, j)
            for j in range(PC):
                load_mask(1, j)

        DRENG = os.environ.get("K_DRENG", "dve,dve,dve,dve,act,act,act,dve").split(",")

        # ---- preamble: E / h / s_dst production ----
        def emit_seg_mm(seg, npieces=2):
            rp = ps.tile([128, 512], dt.float32, tag="bank", name=f"rep{seg}")
            pw = 512 // npieces
            for p in range(npieces):
                for c in range(CC):
                    o = (seg % 2) * 512 + p * pw
                    nc.tensor.matmul(
                        rp[:, p * pw : p * pw + pw],
                        Wsd_t[:, c, F + 1 : F + 2].broadcast_to([128, 128]),
                        xh[seg // 2][:, c, o : o + pw],
                        start=(c == 0),
                        stop=(c == CC - 1),
                    )
            return rp

        def emit_seg_drain(seg, rp):
            half, off = divmod(seg * 512, HALF)
            nc.scalar.activation(
                E2[half][:, off : off + 512], rp[:], AF.Exp,
                bias=cst[:, 0:1], scale=cst[:, 1:2],
            )

        def emit_H_mm(nb):
            # paired h for chunks 2nb, 2nb+1 -> one [128, 512] bank
            hb = ps.tile([128, 512], dt.float32, tag="bank", name=f"H{nb}")
            for half in range(2):
                ch = 2 * nb + half
                q, off = divmod(ch * 128, 1024)
                for c in range(CC):
                    nc.tensor.matmul(
                        hb[:, half * 256 : half * 256 + 256],
                        xh[q][:, c, off : off + 128],
                        Wsd_t[:, c, 0:256],
                        start=(c == 0),
                        stop=(c == CC - 1),
                    )
            return hb

        def emit_sdb_mms(sdb, ch):
            q, off = divmod(ch * 128, 1024)
            for c in range(CC):
                nc.tensor.matmul(
                    sdb[:, ch : ch + 1],
                    xh[q][:, c, off : off + 128],
                    Wsd_t[:, c, F : F + 1],
                    start=(c == 0),
                    stop=(c == CC - 1),
                )

        def emit_hh_drain(nb, hb, eng):
            # GPSIMD cannot read PSUM -- ACT/DVE only here.
            dst = hh[:, 2 * nb : 2 * nb + 2, 0:F]
            if eng == "act":
                nc.scalar.copy(dst, hb[:])
            else:
                nc.vector.tensor_copy(dst, hb[:])

        def emit_fg(sdb, lo, hi):
            nc.scalar.activation(
                Fv[:, lo:hi], sdb[:, lo:hi], AF.Exp, bias=cst[:, 0:1], scale=cst[:, 2:3],
            )
            nc.scalar.activation(
                Gv[:, lo:hi], sdb[:, lo:hi], AF.Exp, bias=cst[:, 0:1], scale=cst[:, 3:4],
            )

        # x quarters 0,1: chunks 0-7, segs 0-1; sdb is long-lived.  The F/G
        # exps ride ACT before the E2 seg drains (they only need the free
        # sdb matmuls) so the DVE strip stream starts as early as possible.
        H = {}
        H[0] = emit_H_mm(0)
        H[1] = emit_H_mm(1)
        rp0 = emit_seg_mm(0)
        load_rest()
        sdb = ps.tile([128, PC], dt.float32, tag="bank", name="sdb")
        for ch in range(0, 8):
            emit_sdb_mms(sdb, ch)
        emit_fg(sdb, 0, 8)
        emit_hh_drain(0, H[0], DRENG[0])
        rp1 = emit_seg_mm(1)
        H[2] = emit_H_mm(2)
        H[3] = emit_H_mm(3)
        emit_seg_drain(0, rp0)
        emit_seg_drain(1, rp1)
        emit_hh_drain(1, H[1], DRENG[1])
        # H2/H3 hh drains ride inside wave 0 (hooks) so the DVE is free to
        # bank score strips as soon as Fv/Gv land

        # x quarter 2: seg 2, chunks 8-11
        rp2 = emit_seg_mm(2)
        emit_seg_drain(2, rp2)
        H[4] = emit_H_mm(4)
        H[5] = emit_H_mm(5)
        for ch in range(8, 12):
            emit_sdb_mms(sdb, ch)

        # x quarter 3: seg 3, s_dst 12-15, chunks 12-15
        rp3 = emit_seg_mm(3)
        for ch in range(12, 16):
            emit_sdb_mms(sdb, ch)
        emit_fg(sdb, 8, 16)
        emit_seg_drain(3, rp3)
        H[6] = emit_H_mm(6)
        H[7] = emit_H_mm(7)
        emit_hh_drain(4, H[4], DRENG[4])
        emit_hh_drain(6, H[6], DRENG[6])
        emit_hh_drain(5, H[5], DRENG[5])
        # hh drain for bank 7 rides inside wave 1 on the DVE (below)

        # ---- score strips ----
        def _env_ks(name, default):
            v = os.environ.get(name)
            if v is None:
                return default
            return tuple(int(t) for t in v.split(",") if t != "")

        POOL_KS = (_env_ks("K_PK0", (2, 5, 8, 11)), _env_ks("K_PK1", (4, 8, 12)))
        # strips whose SECOND half rides Pool while the first half stays DVE
        POOL_H1 = (_env_ks("K_PH0", ()), _env_ks("K_PH1", ()))

        SPLIT_KS = (_env_ks("K_SPLIT0", ()), _env_ks("K_SPLIT1", ()))

        def make_ts(w, k):
            t = tpool.tile([128, HALF], dt.bfloat16, tag="t", name=f"t{w}_{k}")
            if k in SPLIT_KS[w]:
                # half-granular ops: the first half only needs E2 cols 0:512
                # (seg drain 0), letting banks 0-3 start before the second
                # seg drain lands
                nc.vector.tensor_scalar(
                    t[:, 0:512], E2[w][:, 0:512], Fv[:, k : k + 1], Gv[:, k : k + 1],
                    op0=ALU.max, op1=ALU.mult,
                )
                nc.vector.tensor_scalar(
                    t[:, 512:1024], E2[w][:, 512:1024], Fv[:, k : k + 1], Gv[:, k : k + 1],
                    op0=ALU.max, op1=ALU.mult,
                )
            else:
                nc.vector.tensor_scalar(
                    t[:], E2[w][:], Fv[:, k : k + 1], Gv[:, k : k + 1],
                    op0=ALU.max, op1=ALU.mult,
                )
            return t

        def make_ph(w, k, t):
            ph = phpool.tile([128, HALF], dt.bfloat16, tag="ph", name=f"ph{w}_{k}")
            if k in POOL_KS[w]:
                # halves: the strip's first 4 bank-matmuls only need cols 0:512
                nc.gpsimd.tensor_tensor(ph[:, 0:512], t[:, 0:512], maskS[w * PC + k][:, 0:512], op=ALU.mult)
                nc.gpsimd.tensor_tensor(ph[:, 512:1024], t[:, 512:1024], maskS[w * PC + k][:, 512:1024], op=ALU.mult)
            elif k in POOL_H1[w]:
                # first half on the fast DVE (feeds the strip's first 4 bank
                # matmuls), second half on the otherwise-idle GPSIMD
                nc.vector.tensor_mul(ph[:, 0:512], t[:, 0:512], maskS[w * PC + k][:, 0:512])
                nc.gpsimd.tensor_tensor(ph[:, 512:1024], t[:, 512:1024], maskS[w * PC + k][:, 512:1024], op=ALU.mult)
            elif k in SPLIT_KS[w]:
                nc.vector.tensor_mul(ph[:, 0:512], t[:, 0:512], maskS[w * PC + k][:, 0:512])
                nc.vector.tensor_mul(ph[:, 512:1024], t[:, 512:1024], maskS[w * PC + k][:, 512:1024])
            else:
                nc.vector.tensor_mul(ph[:], t[:], maskS[w * PC + k][:])
            return ph

        def emit_mms(banks, ph, k, order=None):
            for ic in (order if order is not None else range(8)):
                nc.tensor.matmul(
                    banks[ic][:, 0 : F + 1],
                    ph[:, ic * 128 : (ic + 1) * 128],
                    hh[:, k, 0 : F + 1],
                    start=(k == 0),
                    stop=(k == PC - 1),
                )

        def emit_norm(ysb, sl, bank, eng):
            rec = spool.tile([128, 1], dt.float32, tag="rec")
            nc.vector.reciprocal(rec[:], bank[:, F : F + 1])
            if eng == "act":
                nc.scalar.activation(ysb[:, sl, :], bank[:, 0:F], AF.Copy, bias=0.0, scale=rec[:, 0:1])
            else:
                nc.vector.tensor_scalar_mul(ysb[:, sl, :], bank[:, 0:F], rec[:, 0:1])

        def emit_wave(w, ybanks, k0_order=None, dve_hooks=None):
            """j-strips 0..PC-1; last STAG strips bank-major with fused norms+stores."""
            t_next = make_ts(w, 0)
            for k in range(PC - STAG):
                t = t_next
                t_next = make_ts(w, k + 1)
                ph = make_ph(w, k, t)
                if dve_hooks and k in dve_hooks:
                    dve_hooks[k]()
                emit_mms(ybanks, ph, k, order=(k0_order if k == 0 else None))
            phs = {}
            for k in range(PC - STAG, PC):
                t = t_next
                if k + 1 < PC:
                    t_next = make_ts(w, k + 1)
                phs[k] = make_ph(w, k, t)
            def _env_groups(name, default):
                v = os.environ.get(name)
                if v is None:
                    return default
                return [tuple(int(t) for t in g.split(":")) for g in v.split(",")]

            groups = (_env_groups("K_GROUPS0", [(0, 2), (2, 2), (4, 2), (6, 2)]) if w == 0
                      else _env_groups("K_GROUPS1", [(0, 4), (4, 3), (7, 1)]))
            ysbs = {g[0]: ypool.tile([128, g[1], F], dt.bfloat16, tag="ysb", name=f"ysb{w}_{g[0]}")
                    for g in groups}
            gof = {}
            for g0, gn in groups:
                for i in range(g0, g0 + gn):
                    gof[i] = (g0, gn)
            # wave 0's scale-copies all ride ACT so the DVE stream rolls
            # straight into wave 1's strips; the final wave alternates.
            engs0 = tuple(os.environ.get("K_ENGS0", "act").split(","))
            engs1 = tuple(os.environ.get("K_ENGS1", "dve,act").split(","))
            engs = engs0 if w == 0 else engs1
            for ic in range(8):
                for k in range(PC - STAG, PC):
                    nc.tensor.matmul(
                        ybanks[ic][:, 0 : F + 1],
                        phs[k][:, ic * 128 : (ic + 1) * 128],
                        hh[:, k, 0 : F + 1],
                        start=False,
                        stop=(k == PC - 1),
                    )
                g0, gn = gof[ic]
                emit_norm(ysbs[g0], ic - g0, ybanks[ic], engs[ic % len(engs)])
                if ic == g0 + gn - 1:
                    lo = w * HALF + g0 * 128
                    nc.sync.dma_start(
                        y[lo : lo + gn * 128, :].rearrange("(c p) f -> p c f", p=128),
                        ysbs[g0][:],
                    )

        # wave 1 -- k0 banks ordered by PSUM-slot drain readiness; hh drains for
        # H5/H7 ride the DVE between early strips
        ybanks1 = [
            ps.tile([128, F + 2], dt.float32, tag="bank", name=f"yb0_{i}")
            for i in range(8)
        ]
        # ps slot children (alloc order warm,H0,H1,rep0,sdb,rep1,H2,H3,rep2,H4,H5,rep3,H6,H7):
        # yb0_0<-H2, yb0_1<-H3, yb0_2<-rep2, yb0_3<-H4, yb0_4<-H5, yb0_5<-rep3, yb0_6<-H6, yb0_7<-H7
        hooks = {
            1: lambda: emit_hh_drain(2, H[2], DRENG[2]),
            2: lambda: emit_hh_drain(3, H[3], DRENG[3]),
            3: lambda: emit_hh_drain(7, H[7], DRENG[7]),
        }
        _k0 = os.environ.get("K_K0", "0,2,1,3,5,6,4,7")
        emit_wave(0, ybanks1, k0_order=[int(t) for t in _k0.split(",")], dve_hooks=hooks)
        ybanks2 = [
            ps.tile([128, F + 2], dt.float32, tag="bank", name=f"yb1_{i}")
            for i in range(8)
        ]
        emit_wave(1, ybanks2)

    nc.compile()
    _CACHE["nc"] = nc
    return nc


def _prep_inputs(x, A, W, a):
    """Host-side layout/precision transforms (per batch element)."""
    import ml_dtypes

    W32 = np.asarray(W, dtype=np.float32)
    a32 = np.asarray(a, dtype=np.float32)
    w_src = W32 @ a32[:F]
    w_dst = W32 @ a32[F:]
    Wsd = np.concatenate([W32, w_dst[:, None], w_src[:, None]], axis=1).astype(np.float16)
    in_maps = []
    for b in range(B):
        xTb = np.asarray(x[b], dtype=np.float32).T.astype(np.float16)
        xWb = np.ascontiguousarray(np.concatenate([Wsd, xTb], axis=1))
        maskTb = np.ascontiguousarray((np.asarray(A[b]).T > 0).astype(ml_dtypes.bfloat16))
        in_maps.append({"xW": xWb, "maskT": maskTb})
    return in_maps


def kernel(x, A, W, a):
    from concourse.bass_utils import run_bass_kernel_spmd

    nc = _build()
    in_maps = _prep_inputs(x, A, W, a)
    res = run_bass_kernel_spmd(nc, in_maps, list(range(B)))
    out = np.stack([np.asarray(res.results[b]["y"]).astype(np.float32) for b in range(B)])
    return out

